# revision 30
# baseline (speedup 1.0000x reference)
"""MoE top-2-of-8 SwiGLU kernel for 8 Trainium2 NeuronCores.

Strategy (expert-parallel with split experts, per sharding hint):
  - Router (tiny: N x E x D matmul) + top-2 dispatch computed on host in
    float64; this IS the sharding step — tokens are gathered per expert id.
  - Each expert's token list is split in half across two cores; each core
    hosts halves of two experts (largest paired with smallest), so the
    per-core capacity is ~max_pair_sum/2 instead of max expert count.
  - Device kernel per core, per expert-slot: h = silu(x@Wg^T) * (x@Wu^T);
    y = h@Wd^T. All matmuls fp16 operands (1 cycle/row on the PE array)
    accumulating in fp32 PSUM. Activations kept transposed
    [feature, token] so the token dim is the moving operand (no 128-row
    quantization of the token count).
  - Host combines: out[n] += w[n,k] * y_row (scatter by the dispatch
    permutation; indices within a slot are unique).

Shapes (hardcoded per problem spec): B=2, S=2048, D=1024, H=4096, E=8, K=2.
"""

import sys

import numpy as np

import concourse.bass as bass
import concourse.tile as tile
from concourse import bacc, mybir
from concourse.bass_utils import run_bass_kernel_spmd


def _ensure_ntff_hook():
    """bass_utils' trace=True path imports antenv.axon_hooks, which some
    agent images lack. Provide the same ctypes shim trn_boot.py would
    install so tracing degrades gracefully instead of crashing."""
    try:
        import antenv.axon_hooks  # noqa: F401
        return
    except ImportError:
        pass
    import contextlib
    import ctypes
    import os
    import types

    so_path = "/opt/axon/libaxon_pjrt.so"
    hook = None
    if os.path.exists(so_path):
        try:
            lib = ctypes.CDLL(so_path)
            if hasattr(lib, "axon_start_nrt_profile"):
                lib.axon_start_nrt_profile.argtypes = [
                    ctypes.POINTER(ctypes.c_int64), ctypes.c_size_t]
                lib.axon_start_nrt_profile.restype = ctypes.c_int64
                lib.axon_stop_nrt_profile.argtypes = [ctypes.c_char_p]
                lib.axon_stop_nrt_profile.restype = ctypes.c_int64

                @contextlib.contextmanager
                def _hook(output_dir, device_ids):
                    import jax
                    jax.devices()
                    if device_ids:
                        ids = (ctypes.c_int64 * len(device_ids))(*device_ids)
                        rc = lib.axon_start_nrt_profile(ids, len(device_ids))
                    else:
                        rc = lib.axon_start_nrt_profile(None, 0)
                    if rc != 0:
                        raise RuntimeError(f"axon_start_nrt_profile rc={rc}")
                    try:
                        yield
                    finally:
                        lib.axon_stop_nrt_profile(str(output_dir).encode())

                hook = _hook
        except OSError:
            hook = None
    mod = types.ModuleType("antenv.axon_hooks")
    mod.get_axon_ntff_profile_hook = lambda: hook
    mod.set_axon_ntff_profile_hook = lambda h: None
    sys.modules["antenv.axon_hooks"] = mod


_ensure_ntff_hook()

TOPK = 2
D = 1024
H = 4096
E = 8
NCORES = 8
DC = D // 128   # 8 contraction chunks of D
HT = H // 128   # 32 tiles of H

_compiled_cache = {}
LAST_RUN = None  # BassKernelResults of the most recent SPMD launch


def _chunk_sizes(C, max_chunk=512):
    """Split C into chunks <= max_chunk (PSUM bank = 512 fp32), multiples
    of 2, near-even."""
    assert C % 2 == 0
    n = -(-C // max_chunk)
    base = -(-C // (2 * n)) * 2
    sizes = []
    left = C
    for _ in range(n):
        s = min(base, left)
        sizes.append(s)
        left -= s
    assert left == 0
    return [s for s in sizes if s]


def _build_kernel(cA, cB, silu_mode="silu"):
    """One SPMD program: two expert slots per core (capacities cA, cB).
    Inputs: xT [128, DC, cA+cB] fp16 (slot A tokens then slot B tokens),
    per-slot pre-tiled weights. Output yT [128, DC, cA+cB] fp32.

    silu_mode="sigmoid_mul" avoids the Silu LUT (not implemented in
    CoreSim) by computing sigmoid on ACT and an extra multiply on DVE.
    """
    C = cA + cB
    parts = [(0, cA, "A"), (cA, cB, "B")]
    f16 = mybir.dt.float16
    f32 = mybir.dt.float32

    nc = bacc.Bacc("TRN2", target_bir_lowering=False, debug=False,
                   num_devices=NCORES)

    xT_d = nc.dram_tensor("xT", [128, DC, C], f16, kind="ExternalInput")
    wdecl = {}
    for _, _, s in parts:
        wdecl["wg" + s] = nc.dram_tensor(
            "wg" + s, [128, HT, DC, 128], f16, kind="ExternalInput")
        wdecl["wu" + s] = nc.dram_tensor(
            "wu" + s, [128, HT, DC, 128], f16, kind="ExternalInput")
        wdecl["wd" + s] = nc.dram_tensor(
            "wd" + s, [128, DC, HT, 128], f16, kind="ExternalInput")
    y_d = nc.dram_tensor("y", [128, DC, C], f32, kind="ExternalOutput")

    with tile.TileContext(nc) as tc:
        with (
            tc.tile_pool(name="xp", bufs=1) as xp,
            tc.tile_pool(name="hp", bufs=1) as hp,
            tc.tile_pool(name="wgp", bufs=3) as wgp,
            tc.tile_pool(name="wup", bufs=3) as wup,
            tc.tile_pool(name="tmp", bufs=3) as tmpp,
            tc.tile_pool(name="outp", bufs=4) as outp,
            tc.tile_pool(name="warm", bufs=1) as warmp,
            tc.tile_pool(name="pa", bufs=2, space="PSUM") as pap,
            tc.tile_pool(name="pb", bufs=2, space="PSUM") as pbp,
            tc.tile_pool(name="pc", bufs=2, space="PSUM") as pcp,
            tc.tile_pool(name="pwarm", bufs=1, space="PSUM") as pwp,
        ):
            # PE warm-up: dummy matmuls on a zeroed scratch tile bridge the
            # initial input-DMA window (~16us, bound by total queued DMA
            # bytes) so the HAM clock-gate opens (1.2->2.4 GHz) before the
            # first real matmul and does not re-cool.
            wsrc = warmp.tile([128, 512], f16)
            nc.gpsimd.memset(wsrc[:], 0)
            pw = pwp.tile([128, 512], f32)
            for _ in range(15):
                nc.tensor.matmul(pw[:], wsrc[:, :128], wsrc[:], start=True,
                                 stop=True)

            # Startup-critical DMAs first: slot-A first weight tiles, then
            # slot-A x split per-dc across sync+gpsimd (SWDGE trigger
            # ~0.6us each). Slot-B x is deferred into the phase-1A loop —
            # it is needed only ~150us in and would otherwise dilute the
            # startup DMA batch (concurrent DMAs complete together).
            wgA0 = wgp.tile([128, DC, 128], f16, tag="w")
            nc.sync.dma_start(wgA0[:], wdecl["wgA"][:, 0])
            wuA0 = wup.tile([128, DC, 128], f16)
            nc.sync.dma_start(wuA0[:], wdecl["wuA"][:, 0])
            xT = xp.tile([128, DC, C], f16)
            for dc in range(DC):
                eng = nc.sync if dc % 2 == 0 else nc.gpsimd
                eng.dma_start(xT[:, dc, 0:cA], xT_d[:, dc, 0:cA])
            hT = hp.tile([128, HT, C], f16)

            # Phase 1 per slot: hT[:, ht, slot] = silu(x@Wg^T) * (x@Wu^T).
            # wg and wd share one pool tag: the wd (and slot-B) prefetch
            # DMAs wait on late slot releases instead of stealing HBM
            # bandwidth from the startup-critical loads.
            for base, cap, s in parts:
                wg_d, wu_d = wdecl["wg" + s], wdecl["wu" + s]
                chunks = _chunk_sizes(cap)
                for ht in range(HT):
                    if s == "A" and ht == 8:
                        # deferred slot-B x load, clear of the startup batch
                        for dc in range(DC):
                            eng = nc.sync if dc % 2 == 0 else nc.gpsimd
                            eng.dma_start(xT[:, dc, cA:C], xT_d[:, dc, cA:C])
                    if s == "A" and ht == 0:
                        wg, wu = wgA0, wuA0
                    else:
                        wg = wgp.tile([128, DC, 128], f16, tag="w")
                        nc.sync.dma_start(wg[:], wg_d[:, ht])
                        wu = wup.tile([128, DC, 128], f16)
                        nc.sync.dma_start(wu[:], wu_d[:, ht])
                    sl = tmpp.tile([128, cap], f32, tag="sl")
                    t0 = base
                    for tn in chunks:
                        pA = pap.tile([128, tn], f32, tag="pA")
                        for dc in range(DC):
                            nc.tensor.matmul(pA[:, 0:tn], wg[:, dc],
                                             xT[:, dc, t0:t0 + tn],
                                             start=(dc == 0),
                                             stop=(dc == DC - 1))
                        pB = pbp.tile([128, tn], f32, tag="pB")
                        for dc in range(DC):
                            nc.tensor.matmul(pB[:, 0:tn], wu[:, dc],
                                             xT[:, dc, t0:t0 + tn],
                                             start=(dc == 0),
                                             stop=(dc == DC - 1))
                        slc = sl[:, t0 - base:t0 - base + tn]
                        if silu_mode == "silu":
                            nc.scalar.activation(
                                slc, pA[:, 0:tn],
                                mybir.ActivationFunctionType.Silu)
                        else:
                            nc.scalar.activation(
                                slc, pA[:, 0:tn],
                                mybir.ActivationFunctionType.Sigmoid)
                            nc.vector.tensor_mul(slc, slc, pA[:, 0:tn])
                        nc.vector.tensor_mul(hT[:, ht, t0:t0 + tn], slc,
                                             pB[:, 0:tn])
                        t0 += tn

            # Phase 2 per slot: y[:, dt, slot] = h @ Wd^T.
            for base, cap, s in parts:
                wd_d = wdecl["wd" + s]
                chunks = _chunk_sizes(cap)
                for dt in range(DC):
                    wd = wgp.tile([128, HT, 128], f16, tag="w")
                    nc.sync.dma_start(wd[:], wd_d[:, dt])
                    ot = outp.tile([128, cap], f32, tag="ot")
                    t0 = base
                    for tn in chunks:
                        pC = pcp.tile([128, tn], f32, tag="pC")
                        for hc in range(HT):
                            nc.tensor.matmul(pC[:, 0:tn], wd[:, hc],
                                             hT[:, hc, t0:t0 + tn],
                                             start=(hc == 0),
                                             stop=(hc == HT - 1))
                        oc = ot[:, t0 - base:t0 - base + tn]
                        nc.vector.tensor_copy(oc, pC[:, 0:tn])
                        nc.sync.dma_start(y_d[:, dt, t0:t0 + tn], oc)
                        t0 += tn

    nc.compile()
    return nc


def _get_kernel(cA, cB):
    key = (cA, cB)
    if key not in _compiled_cache:
        _compiled_cache[key] = _build_kernel(cA, cB)
    return _compiled_cache[key]


def _route(xt, Wr):
    """Host router in float64: logits, top-2 (desc, ties by index like
    jax.lax.top_k), renormalized weights."""
    logits64 = xt.astype(np.float64) @ Wr.T.astype(np.float64)
    m = logits64.max(axis=-1, keepdims=True)
    p = np.exp(logits64 - m)
    p /= p.sum(axis=-1, keepdims=True)
    idx = np.argsort(-p, axis=-1, kind="stable")[:, :TOPK]
    w = np.take_along_axis(p, idx, axis=-1)
    w /= w.sum(axis=-1, keepdims=True)
    return logits64, idx, w.astype(np.float32)


def _tile_x(xe):
    """[C, D] fp16 -> [128, DC, C]: [p, dc, t] = xe[t, dc*128+p]."""
    Cc = xe.shape[0]
    return np.ascontiguousarray(xe.T.reshape(DC, 128, Cc).transpose(1, 0, 2))


def _tile_wgwu(w):
    """[H, D] -> [128, HT, DC, 128] fp16: [p,ht,dc,m] = w[ht*128+m, dc*128+p]."""
    return np.ascontiguousarray(
        w.reshape(HT, 128, DC, 128).transpose(3, 0, 2, 1).astype(np.float16))


def _tile_wd(w):
    """[D, H] -> [128, DC, HT, 128] fp16: [p,dt,hc,m] = w[dt*128+m, hc*128+p]."""
    return np.ascontiguousarray(
        w.reshape(DC, 128, HT, 128).transpose(3, 0, 2, 1).astype(np.float16))


def _r2(n):
    return max(8, -(-n // 2) * 2)


def kernel(x, Wr, Wg, Wu, Wd):
    B, S, _ = x.shape
    N = B * S
    xt = np.ascontiguousarray(np.asarray(x, dtype=np.float32).reshape(N, D))
    Wr = np.asarray(Wr, dtype=np.float32)

    logits64, idx, w = _route(xt, Wr)

    # dispatch lists per expert (np.where on [N, K] is token-ordered)
    rows_n, rows_k, counts = [], [], []
    for e in range(E):
        rn, rk = np.nonzero(idx == e)
        rows_n.append(rn)
        rows_k.append(rk)
        counts.append(len(rn))
    counts = np.asarray(counts)

    # pair largest with smallest expert; each pair spans two cores
    order = np.argsort(-counts, kind="stable")
    pairs = [(int(order[i]), int(order[E - 1 - i])) for i in range(E // 2)]
    cA = _r2(max(-(-counts[eb] // 2) for eb, _ in pairs))
    cB = _r2(max(-(-counts[es] // 2) for _, es in pairs))

    nc = _get_kernel(cA, cB)

    xt16 = xt.astype(np.float16)
    wts = {e: (_tile_wgwu(np.asarray(Wg[e])), _tile_wgwu(np.asarray(Wu[e])),
               _tile_wd(np.asarray(Wd[e]))) for e in range(E)}

    # slot assignment: pair p -> cores 2p, 2p+1; core k gets half k of each
    # expert's token list (slot A = big expert, slot B = small expert)
    slot_rows = []  # per core: (rowsA, rowsB) token/slot index arrays
    in_maps = []
    for p, (eb, es) in enumerate(pairs):
        hb = (counts[eb] + 1) // 2
        hs = (counts[es] + 1) // 2
        for k in range(2):
            ra = slice(0, hb) if k == 0 else slice(hb, counts[eb])
            rb = slice(0, hs) if k == 0 else slice(hs, counts[es])
            rnA, rkA = rows_n[eb][ra], rows_k[eb][ra]
            rnB, rkB = rows_n[es][rb], rows_k[es][rb]
            xe = np.zeros((cA + cB, D), dtype=np.float16)
            xe[:len(rnA)] = xt16[rnA]
            xe[cA:cA + len(rnB)] = xt16[rnB]
            gA, uA, dA = wts[eb]
            gB, uB, dB = wts[es]
            in_maps.append({"xT": _tile_x(xe),
                            "wgA": gA, "wuA": uA, "wdA": dA,
                            "wgB": gB, "wuB": uB, "wdB": dB})
            slot_rows.append(((rnA, rkA), (rnB, rkB)))

    global LAST_RUN
    LAST_RUN = run_bass_kernel_spmd(nc, in_maps, list(range(NCORES)))
    results = LAST_RUN.results

    out = np.zeros((N, D), dtype=np.float32)
    for c in range(NCORES):
        y_t = results[c]["y"]                       # [128, DC, cA+cB] f32
        y_tok = y_t.transpose(2, 1, 0).reshape(cA + cB, D)
        for (rn, rk), base in zip(slot_rows[c], (0, cA)):
            if len(rn) == 0:
                continue
            out[rn] += y_tok[base:base + len(rn)] * w[rn, rk][:, None]

    return out.reshape(B, S, D), logits64.astype(np.float32)


# revision 34
# speedup vs baseline: 1.0039x; 1.0039x over previous
"""MoE top-2-of-8 SwiGLU kernel for 8 Trainium2 NeuronCores.

Strategy (expert-parallel with split experts, per sharding hint):
  - Router (tiny: N x E x D matmul) + top-2 dispatch computed on host in
    float64; this IS the sharding step — tokens are gathered per expert id.
  - Each expert's token list is split in half across two cores; each core
    hosts halves of two experts (largest paired with smallest), so the
    per-core capacity is ~max_pair_sum/2 instead of max expert count.
  - Device kernel per core, per expert-slot: h = silu(x@Wg^T) * (x@Wu^T);
    y = h@Wd^T. All matmuls fp16 operands (1 cycle/row on the PE array)
    accumulating in fp32 PSUM. Activations kept transposed
    [feature, token] so the token dim is the moving operand (no 128-row
    quantization of the token count).
  - Host combines: out[n] += w[n,k] * y_row (scatter by the dispatch
    permutation; indices within a slot are unique).

Shapes (hardcoded per problem spec): B=2, S=2048, D=1024, H=4096, E=8, K=2.
"""

import sys

import numpy as np

import concourse.bass as bass
import concourse.tile as tile
from concourse import bacc, mybir
from concourse.bass_utils import run_bass_kernel_spmd


def _ensure_ntff_hook():
    """bass_utils' trace=True path imports antenv.axon_hooks, which some
    agent images lack. Provide the same ctypes shim trn_boot.py would
    install so tracing degrades gracefully instead of crashing."""
    try:
        import antenv.axon_hooks  # noqa: F401
        return
    except ImportError:
        pass
    import contextlib
    import ctypes
    import os
    import types

    so_path = "/opt/axon/libaxon_pjrt.so"
    hook = None
    if os.path.exists(so_path):
        try:
            lib = ctypes.CDLL(so_path)
            if hasattr(lib, "axon_start_nrt_profile"):
                lib.axon_start_nrt_profile.argtypes = [
                    ctypes.POINTER(ctypes.c_int64), ctypes.c_size_t]
                lib.axon_start_nrt_profile.restype = ctypes.c_int64
                lib.axon_stop_nrt_profile.argtypes = [ctypes.c_char_p]
                lib.axon_stop_nrt_profile.restype = ctypes.c_int64

                @contextlib.contextmanager
                def _hook(output_dir, device_ids):
                    import jax
                    jax.devices()
                    if device_ids:
                        ids = (ctypes.c_int64 * len(device_ids))(*device_ids)
                        rc = lib.axon_start_nrt_profile(ids, len(device_ids))
                    else:
                        rc = lib.axon_start_nrt_profile(None, 0)
                    if rc != 0:
                        raise RuntimeError(f"axon_start_nrt_profile rc={rc}")
                    try:
                        yield
                    finally:
                        lib.axon_stop_nrt_profile(str(output_dir).encode())

                hook = _hook
        except OSError:
            hook = None
    mod = types.ModuleType("antenv.axon_hooks")
    mod.get_axon_ntff_profile_hook = lambda: hook
    mod.set_axon_ntff_profile_hook = lambda h: None
    sys.modules["antenv.axon_hooks"] = mod


_ensure_ntff_hook()

TOPK = 2
D = 1024
H = 4096
E = 8
NCORES = 8
DC = D // 128   # 8 contraction chunks of D
HT = H // 128   # 32 tiles of H

_compiled_cache = {}
LAST_RUN = None  # BassKernelResults of the most recent SPMD launch


def _chunk_sizes(C, max_chunk=512):
    """Split C into chunks <= max_chunk (PSUM bank = 512 fp32), multiples
    of 2, near-even."""
    assert C % 2 == 0
    n = -(-C // max_chunk)
    base = -(-C // (2 * n)) * 2
    sizes = []
    left = C
    for _ in range(n):
        s = min(base, left)
        sizes.append(s)
        left -= s
    assert left == 0
    return [s for s in sizes if s]


def _build_kernel(cA, cB, silu_mode="silu"):
    """One SPMD program: two expert slots per core (capacities cA, cB).
    Inputs: xT [128, DC, cA+cB] fp16 (slot A tokens then slot B tokens),
    per-slot pre-tiled weights. Output yT [128, DC, cA+cB] fp32.

    silu_mode="sigmoid_mul" avoids the Silu LUT (not implemented in
    CoreSim) by computing sigmoid on ACT and an extra multiply on DVE.
    """
    C = cA + cB
    parts = [(0, cA, "A"), (cA, cB, "B")]
    f16 = mybir.dt.float16
    f32 = mybir.dt.float32

    nc = bacc.Bacc("TRN2", target_bir_lowering=False, debug=False,
                   num_devices=NCORES)

    xT_d = nc.dram_tensor("xT", [128, DC, C], f16, kind="ExternalInput")
    wdecl = {}
    for _, _, s in parts:
        wdecl["wg" + s] = nc.dram_tensor(
            "wg" + s, [128, HT, DC, 128], f16, kind="ExternalInput")
        wdecl["wu" + s] = nc.dram_tensor(
            "wu" + s, [128, HT, DC, 128], f16, kind="ExternalInput")
        wdecl["wd" + s] = nc.dram_tensor(
            "wd" + s, [128, DC, HT, 128], f16, kind="ExternalInput")
    y_d = nc.dram_tensor("y", [128, DC, C], f32, kind="ExternalOutput")

    with tile.TileContext(nc) as tc:
        with (
            tc.tile_pool(name="xp", bufs=1) as xp,
            tc.tile_pool(name="hp", bufs=1) as hp,
            tc.tile_pool(name="wgp", bufs=3) as wgp,
            tc.tile_pool(name="wup", bufs=3) as wup,
            tc.tile_pool(name="tmp", bufs=3) as tmpp,
            tc.tile_pool(name="outp", bufs=4) as outp,
            tc.tile_pool(name="warm", bufs=1) as warmp,
            tc.tile_pool(name="pa", bufs=2, space="PSUM") as pap,
            tc.tile_pool(name="pb", bufs=2, space="PSUM") as pbp,
            tc.tile_pool(name="pc", bufs=2, space="PSUM") as pcp,
            tc.tile_pool(name="pwarm", bufs=1, space="PSUM") as pwp,
        ):
            # PE warm-up: dummy matmuls on a zeroed scratch tile bridge the
            # initial input-DMA window (~16us, bound by total queued DMA
            # bytes) so the HAM clock-gate opens (1.2->2.4 GHz) before the
            # first real matmul and does not re-cool.
            wsrc = warmp.tile([128, 512], f16)
            nc.gpsimd.memset(wsrc[:], 0)
            pw = pwp.tile([128, 512], f32)
            for _ in range(14):
                nc.tensor.matmul(pw[:], wsrc[:, :128], wsrc[:], start=True,
                                 stop=True)

            # Startup-critical DMAs first: slot-A first weight tiles, then
            # slot-A x split per-dc across sync+gpsimd (SWDGE trigger
            # ~0.6us each). Slot-B x is deferred into the phase-1A loop —
            # it is needed only ~150us in and would otherwise dilute the
            # startup DMA batch (concurrent DMAs complete together).
            wgA0 = wgp.tile([128, DC, 128], f16, tag="w")
            nc.sync.dma_start(wgA0[:], wdecl["wgA"][:, 0])
            wuA0 = wup.tile([128, DC, 128], f16)
            nc.sync.dma_start(wuA0[:], wdecl["wuA"][:, 0])
            xT = xp.tile([128, DC, C], f16)
            h2 = DC // 2
            nc.sync.dma_start(xT[:, 0:h2, 0:cA], xT_d[:, 0:h2, 0:cA])
            nc.gpsimd.dma_start(xT[:, h2:DC, 0:cA], xT_d[:, h2:DC, 0:cA])
            hT = hp.tile([128, HT, C], f16)

            # Phase 1 per slot: hT[:, ht, slot] = silu(x@Wg^T) * (x@Wu^T).
            # wg and wd share one pool tag: the wd (and slot-B) prefetch
            # DMAs wait on late slot releases instead of stealing HBM
            # bandwidth from the startup-critical loads.
            for base, cap, s in parts:
                wg_d, wu_d = wdecl["wg" + s], wdecl["wu" + s]
                chunks = _chunk_sizes(cap)
                for ht in range(HT):
                    if s == "A" and ht == 8:
                        # deferred slot-B x load, clear of the startup batch
                        nc.sync.dma_start(xT[:, 0:h2, cA:C],
                                          xT_d[:, 0:h2, cA:C])
                        nc.gpsimd.dma_start(xT[:, h2:DC, cA:C],
                                            xT_d[:, h2:DC, cA:C])
                    if s == "A" and ht == 0:
                        wg, wu = wgA0, wuA0
                    else:
                        wg = wgp.tile([128, DC, 128], f16, tag="w")
                        nc.sync.dma_start(wg[:], wg_d[:, ht])
                        wu = wup.tile([128, DC, 128], f16)
                        nc.sync.dma_start(wu[:], wu_d[:, ht])
                    sl = tmpp.tile([128, cap], f32, tag="sl")
                    t0 = base
                    for tn in chunks:
                        pA = pap.tile([128, tn], f32, tag="pA")
                        for dc in range(DC):
                            nc.tensor.matmul(pA[:, 0:tn], wg[:, dc],
                                             xT[:, dc, t0:t0 + tn],
                                             start=(dc == 0),
                                             stop=(dc == DC - 1))
                        pB = pbp.tile([128, tn], f32, tag="pB")
                        for dc in range(DC):
                            nc.tensor.matmul(pB[:, 0:tn], wu[:, dc],
                                             xT[:, dc, t0:t0 + tn],
                                             start=(dc == 0),
                                             stop=(dc == DC - 1))
                        slc = sl[:, t0 - base:t0 - base + tn]
                        if silu_mode == "silu":
                            nc.scalar.activation(
                                slc, pA[:, 0:tn],
                                mybir.ActivationFunctionType.Silu)
                        else:
                            nc.scalar.activation(
                                slc, pA[:, 0:tn],
                                mybir.ActivationFunctionType.Sigmoid)
                            nc.vector.tensor_mul(slc, slc, pA[:, 0:tn])
                        nc.vector.tensor_mul(hT[:, ht, t0:t0 + tn], slc,
                                             pB[:, 0:tn])
                        t0 += tn

            # Phase 2 per slot: y[:, dt, slot] = h @ Wd^T. Slot B first so
            # the kernel tail ends on slot A's smaller last chunk.
            for base, cap, s in parts[::-1]:
                wd_d = wdecl["wd" + s]
                chunks = _chunk_sizes(cap)
                for dt in range(DC):
                    wd = wgp.tile([128, HT, 128], f16, tag="w")
                    nc.sync.dma_start(wd[:], wd_d[:, dt])
                    ot = outp.tile([128, cap], f32, tag="ot")
                    t0 = base
                    for tn in chunks:
                        pC = pcp.tile([128, tn], f32, tag="pC")
                        for hc in range(HT):
                            nc.tensor.matmul(pC[:, 0:tn], wd[:, hc],
                                             hT[:, hc, t0:t0 + tn],
                                             start=(hc == 0),
                                             stop=(hc == HT - 1))
                        oc = ot[:, t0 - base:t0 - base + tn]
                        nc.vector.tensor_copy(oc, pC[:, 0:tn])
                        nc.sync.dma_start(y_d[:, dt, t0:t0 + tn], oc)
                        t0 += tn

    nc.compile()
    return nc


def _get_kernel(cA, cB):
    key = (cA, cB)
    if key not in _compiled_cache:
        _compiled_cache[key] = _build_kernel(cA, cB)
    return _compiled_cache[key]


def _route(xt, Wr):
    """Host router in float64: logits, top-2 (desc, ties by index like
    jax.lax.top_k), renormalized weights."""
    logits64 = xt.astype(np.float64) @ Wr.T.astype(np.float64)
    m = logits64.max(axis=-1, keepdims=True)
    p = np.exp(logits64 - m)
    p /= p.sum(axis=-1, keepdims=True)
    idx = np.argsort(-p, axis=-1, kind="stable")[:, :TOPK]
    w = np.take_along_axis(p, idx, axis=-1)
    w /= w.sum(axis=-1, keepdims=True)
    return logits64, idx, w.astype(np.float32)


def _tile_x(xe):
    """[C, D] fp16 -> [128, DC, C]: [p, dc, t] = xe[t, dc*128+p]."""
    Cc = xe.shape[0]
    return np.ascontiguousarray(xe.T.reshape(DC, 128, Cc).transpose(1, 0, 2))


def _tile_wgwu(w):
    """[H, D] -> [128, HT, DC, 128] fp16: [p,ht,dc,m] = w[ht*128+m, dc*128+p]."""
    return np.ascontiguousarray(
        w.reshape(HT, 128, DC, 128).transpose(3, 0, 2, 1).astype(np.float16))


def _tile_wd(w):
    """[D, H] -> [128, DC, HT, 128] fp16: [p,dt,hc,m] = w[dt*128+m, hc*128+p]."""
    return np.ascontiguousarray(
        w.reshape(DC, 128, HT, 128).transpose(3, 0, 2, 1).astype(np.float16))


def _r2(n):
    return max(8, -(-n // 2) * 2)


def kernel(x, Wr, Wg, Wu, Wd):
    B, S, _ = x.shape
    N = B * S
    xt = np.ascontiguousarray(np.asarray(x, dtype=np.float32).reshape(N, D))
    Wr = np.asarray(Wr, dtype=np.float32)

    logits64, idx, w = _route(xt, Wr)

    # dispatch lists per expert (np.where on [N, K] is token-ordered)
    rows_n, rows_k, counts = [], [], []
    for e in range(E):
        rn, rk = np.nonzero(idx == e)
        rows_n.append(rn)
        rows_k.append(rk)
        counts.append(len(rn))
    counts = np.asarray(counts)

    # pair largest with smallest expert; each pair spans two cores
    order = np.argsort(-counts, kind="stable")
    pairs = [(int(order[i]), int(order[E - 1 - i])) for i in range(E // 2)]
    cA = _r2(max(-(-counts[eb] // 2) for eb, _ in pairs))
    cB = _r2(max(-(-counts[es] // 2) for _, es in pairs))

    nc = _get_kernel(cA, cB)

    xt16 = xt.astype(np.float16)
    wts = {e: (_tile_wgwu(np.asarray(Wg[e])), _tile_wgwu(np.asarray(Wu[e])),
               _tile_wd(np.asarray(Wd[e]))) for e in range(E)}

    # slot assignment: pair p -> cores 2p, 2p+1; core k gets half k of each
    # expert's token list (slot A = big expert, slot B = small expert)
    slot_rows = []  # per core: (rowsA, rowsB) token/slot index arrays
    in_maps = []
    for p, (eb, es) in enumerate(pairs):
        hb = (counts[eb] + 1) // 2
        hs = (counts[es] + 1) // 2
        for k in range(2):
            ra = slice(0, hb) if k == 0 else slice(hb, counts[eb])
            rb = slice(0, hs) if k == 0 else slice(hs, counts[es])
            rnA, rkA = rows_n[eb][ra], rows_k[eb][ra]
            rnB, rkB = rows_n[es][rb], rows_k[es][rb]
            xe = np.zeros((cA + cB, D), dtype=np.float16)
            xe[:len(rnA)] = xt16[rnA]
            xe[cA:cA + len(rnB)] = xt16[rnB]
            gA, uA, dA = wts[eb]
            gB, uB, dB = wts[es]
            in_maps.append({"xT": _tile_x(xe),
                            "wgA": gA, "wuA": uA, "wdA": dA,
                            "wgB": gB, "wuB": uB, "wdB": dB})
            slot_rows.append(((rnA, rkA), (rnB, rkB)))

    global LAST_RUN
    LAST_RUN = run_bass_kernel_spmd(nc, in_maps, list(range(NCORES)))
    results = LAST_RUN.results

    out = np.zeros((N, D), dtype=np.float32)
    for c in range(NCORES):
        y_t = results[c]["y"]                       # [128, DC, cA+cB] f32
        y_tok = y_t.transpose(2, 1, 0).reshape(cA + cB, D)
        for (rn, rk), base in zip(slot_rows[c], (0, cA)):
            if len(rn) == 0:
                continue
            out[rn] += y_tok[base:base + len(rn)] * w[rn, rk][:, None]

    return out.reshape(B, S, D), logits64.astype(np.float32)


# revision 36
# speedup vs baseline: 1.0142x; 1.0102x over previous
"""MoE top-2-of-8 SwiGLU kernel for 8 Trainium2 NeuronCores.

Strategy (expert-parallel with split experts, per sharding hint):
  - Router (tiny: N x E x D matmul) + top-2 dispatch computed on host in
    float64; this IS the sharding step — tokens are gathered per expert id.
  - Each expert's token list is split in half across two cores; each core
    hosts halves of two experts (largest paired with smallest), so the
    per-core capacity is ~max_pair_sum/2 instead of max expert count.
  - Device kernel per core, per expert-slot: h = silu(x@Wg^T) * (x@Wu^T);
    y = h@Wd^T. All matmuls fp16 operands (1 cycle/row on the PE array)
    accumulating in fp32 PSUM. Activations kept transposed
    [feature, token] so the token dim is the moving operand (no 128-row
    quantization of the token count).
  - Host combines: out[n] += w[n,k] * y_row (scatter by the dispatch
    permutation; indices within a slot are unique).

Shapes (hardcoded per problem spec): B=2, S=2048, D=1024, H=4096, E=8, K=2.
"""

import sys

import numpy as np

import concourse.bass as bass
import concourse.tile as tile
from concourse import bacc, mybir
from concourse.bass_utils import run_bass_kernel_spmd


def _ensure_ntff_hook():
    """bass_utils' trace=True path imports antenv.axon_hooks, which some
    agent images lack. Provide the same ctypes shim trn_boot.py would
    install so tracing degrades gracefully instead of crashing."""
    try:
        import antenv.axon_hooks  # noqa: F401
        return
    except ImportError:
        pass
    import contextlib
    import ctypes
    import os
    import types

    so_path = "/opt/axon/libaxon_pjrt.so"
    hook = None
    if os.path.exists(so_path):
        try:
            lib = ctypes.CDLL(so_path)
            if hasattr(lib, "axon_start_nrt_profile"):
                lib.axon_start_nrt_profile.argtypes = [
                    ctypes.POINTER(ctypes.c_int64), ctypes.c_size_t]
                lib.axon_start_nrt_profile.restype = ctypes.c_int64
                lib.axon_stop_nrt_profile.argtypes = [ctypes.c_char_p]
                lib.axon_stop_nrt_profile.restype = ctypes.c_int64

                @contextlib.contextmanager
                def _hook(output_dir, device_ids):
                    import jax
                    jax.devices()
                    if device_ids:
                        ids = (ctypes.c_int64 * len(device_ids))(*device_ids)
                        rc = lib.axon_start_nrt_profile(ids, len(device_ids))
                    else:
                        rc = lib.axon_start_nrt_profile(None, 0)
                    if rc != 0:
                        raise RuntimeError(f"axon_start_nrt_profile rc={rc}")
                    try:
                        yield
                    finally:
                        lib.axon_stop_nrt_profile(str(output_dir).encode())

                hook = _hook
        except OSError:
            hook = None
    mod = types.ModuleType("antenv.axon_hooks")
    mod.get_axon_ntff_profile_hook = lambda: hook
    mod.set_axon_ntff_profile_hook = lambda h: None
    sys.modules["antenv.axon_hooks"] = mod


_ensure_ntff_hook()

TOPK = 2
D = 1024
H = 4096
E = 8
NCORES = 8
DC = D // 128   # 8 contraction chunks of D
HT = H // 128   # 32 tiles of H

_compiled_cache = {}
LAST_RUN = None  # BassKernelResults of the most recent SPMD launch


def _chunk_sizes(C, max_chunk=512):
    """Split C into chunks <= max_chunk (PSUM bank = 512 fp32), multiples
    of 2, near-even."""
    assert C % 2 == 0
    n = -(-C // max_chunk)
    base = -(-C // (2 * n)) * 2
    sizes = []
    left = C
    for _ in range(n):
        s = min(base, left)
        sizes.append(s)
        left -= s
    assert left == 0
    return [s for s in sizes if s]


def _build_kernel(cA, cB, silu_mode="silu"):
    """One SPMD program: two expert slots per core (capacities cA, cB).
    Inputs: xT [128, DC, cA+cB] fp16 (slot A tokens then slot B tokens),
    per-slot pre-tiled weights. Output yT [128, DC, cA+cB] fp32.

    silu_mode="sigmoid_mul" avoids the Silu LUT (not implemented in
    CoreSim) by computing sigmoid on ACT and an extra multiply on DVE.
    """
    C = cA + cB
    parts = [(0, cA, "A"), (cA, cB, "B")]
    f16 = mybir.dt.float16
    f32 = mybir.dt.float32

    nc = bacc.Bacc("TRN2", target_bir_lowering=False, debug=False,
                   num_devices=NCORES)

    xT_d = nc.dram_tensor("xT", [128, DC, C], f16, kind="ExternalInput")
    wdecl = {}
    for _, _, s in parts:
        wdecl["wg" + s] = nc.dram_tensor(
            "wg" + s, [128, HT, DC, 128], f16, kind="ExternalInput")
        wdecl["wu" + s] = nc.dram_tensor(
            "wu" + s, [128, HT, DC, 128], f16, kind="ExternalInput")
        wdecl["wd" + s] = nc.dram_tensor(
            "wd" + s, [128, DC, HT, 128], f16, kind="ExternalInput")
    y_d = nc.dram_tensor("y", [128, DC, C], f32, kind="ExternalOutput")

    with tile.TileContext(nc) as tc:
        with (
            tc.tile_pool(name="xp", bufs=1) as xp,
            tc.tile_pool(name="hp", bufs=1) as hp,
            tc.tile_pool(name="wgp", bufs=3) as wgp,
            tc.tile_pool(name="wup", bufs=3) as wup,
            tc.tile_pool(name="tmp", bufs=3) as tmpp,
            tc.tile_pool(name="outp", bufs=4) as outp,
            tc.tile_pool(name="warm", bufs=1) as warmp,
            tc.tile_pool(name="pa", bufs=2, space="PSUM") as pap,
            tc.tile_pool(name="pb", bufs=2, space="PSUM") as pbp,
            tc.tile_pool(name="pc", bufs=2, space="PSUM") as pcp,
            tc.tile_pool(name="pwarm", bufs=1, space="PSUM") as pwp,
        ):
            # PE warm-up: dummy matmuls on a zeroed scratch tile bridge the
            # initial input-DMA window (~16us, bound by total queued DMA
            # bytes) so the HAM clock-gate opens (1.2->2.4 GHz) before the
            # first real matmul and does not re-cool.
            wsrc = warmp.tile([128, 512], f16)
            nc.gpsimd.memset(wsrc[:], 0)
            pw = pwp.tile([128, 512], f32)
            for _ in range(16):
                nc.tensor.matmul(pw[:], wsrc[:, :128], wsrc[:], start=True,
                                 stop=True)

            # Startup-critical DMAs first: slot-A first weight tiles, then
            # slot-A x split per-dc across sync+gpsimd (SWDGE trigger
            # ~0.6us each). Slot-B x is deferred into the phase-1A loop —
            # it is needed only ~150us in and would otherwise dilute the
            # startup DMA batch (concurrent DMAs complete together).
            wgA0 = wgp.tile([128, DC, 128], f16, tag="w")
            nc.sync.dma_start(wgA0[:], wdecl["wgA"][:, 0])
            wuA0 = wup.tile([128, DC, 128], f16)
            nc.sync.dma_start(wuA0[:], wdecl["wuA"][:, 0])
            xT = xp.tile([128, DC, C], f16)
            h2 = DC // 2
            for dc in range(DC):
                eng = nc.sync if dc % 2 == 0 else nc.gpsimd
                eng.dma_start(xT[:, dc, 0:cA], xT_d[:, dc, 0:cA])
            hT = hp.tile([128, HT, C], f16)

            # Phase 1 per slot: hT[:, ht, slot] = silu(x@Wg^T) * (x@Wu^T).
            # wg and wd share one pool tag: the wd (and slot-B) prefetch
            # DMAs wait on late slot releases instead of stealing HBM
            # bandwidth from the startup-critical loads.
            for base, cap, s in parts:
                wg_d, wu_d = wdecl["wg" + s], wdecl["wu" + s]
                chunks = _chunk_sizes(cap)
                for ht in range(HT):
                    if s == "A" and ht == 8:
                        # deferred slot-B x load, clear of the startup batch
                        nc.sync.dma_start(xT[:, 0:h2, cA:C],
                                          xT_d[:, 0:h2, cA:C])
                        nc.gpsimd.dma_start(xT[:, h2:DC, cA:C],
                                            xT_d[:, h2:DC, cA:C])
                    if s == "A" and ht == 0:
                        wg, wu = wgA0, wuA0
                    else:
                        wg = wgp.tile([128, DC, 128], f16, tag="w")
                        nc.sync.dma_start(wg[:], wg_d[:, ht])
                        wu = wup.tile([128, DC, 128], f16)
                        nc.sync.dma_start(wu[:], wu_d[:, ht])
                    sl = tmpp.tile([128, cap], f32, tag="sl")
                    t0 = base
                    for tn in chunks:
                        pA = pap.tile([128, tn], f32, tag="pA")
                        for dc in range(DC):
                            nc.tensor.matmul(pA[:, 0:tn], wg[:, dc],
                                             xT[:, dc, t0:t0 + tn],
                                             start=(dc == 0),
                                             stop=(dc == DC - 1))
                        pB = pbp.tile([128, tn], f32, tag="pB")
                        for dc in range(DC):
                            nc.tensor.matmul(pB[:, 0:tn], wu[:, dc],
                                             xT[:, dc, t0:t0 + tn],
                                             start=(dc == 0),
                                             stop=(dc == DC - 1))
                        slc = sl[:, t0 - base:t0 - base + tn]
                        if silu_mode == "silu":
                            nc.scalar.activation(
                                slc, pA[:, 0:tn],
                                mybir.ActivationFunctionType.Silu)
                        else:
                            nc.scalar.activation(
                                slc, pA[:, 0:tn],
                                mybir.ActivationFunctionType.Sigmoid)
                            nc.vector.tensor_mul(slc, slc, pA[:, 0:tn])
                        nc.vector.tensor_mul(hT[:, ht, t0:t0 + tn], slc,
                                             pB[:, 0:tn])
                        t0 += tn

            # Phase 2 per slot: y[:, dt, slot] = h @ Wd^T. Slot B first so
            # the kernel tail ends on slot A's smaller last chunk.
            for base, cap, s in parts[::-1]:
                wd_d = wdecl["wd" + s]
                chunks = _chunk_sizes(cap)
                for dt in range(DC):
                    wd = wgp.tile([128, HT, 128], f16, tag="w")
                    nc.sync.dma_start(wd[:], wd_d[:, dt])
                    ot = outp.tile([128, cap], f32, tag="ot")
                    t0 = base
                    for tn in chunks:
                        pC = pcp.tile([128, tn], f32, tag="pC")
                        for hc in range(HT):
                            nc.tensor.matmul(pC[:, 0:tn], wd[:, hc],
                                             hT[:, hc, t0:t0 + tn],
                                             start=(hc == 0),
                                             stop=(hc == HT - 1))
                        oc = ot[:, t0 - base:t0 - base + tn]
                        nc.vector.tensor_copy(oc, pC[:, 0:tn])
                        nc.sync.dma_start(y_d[:, dt, t0:t0 + tn], oc)
                        t0 += tn

    nc.compile()
    return nc


def _get_kernel(cA, cB):
    key = (cA, cB)
    if key not in _compiled_cache:
        _compiled_cache[key] = _build_kernel(cA, cB)
    return _compiled_cache[key]


def _route(xt, Wr):
    """Host router in float64: logits, top-2 (desc, ties by index like
    jax.lax.top_k), renormalized weights."""
    logits64 = xt.astype(np.float64) @ Wr.T.astype(np.float64)
    m = logits64.max(axis=-1, keepdims=True)
    p = np.exp(logits64 - m)
    p /= p.sum(axis=-1, keepdims=True)
    idx = np.argsort(-p, axis=-1, kind="stable")[:, :TOPK]
    w = np.take_along_axis(p, idx, axis=-1)
    w /= w.sum(axis=-1, keepdims=True)
    return logits64, idx, w.astype(np.float32)


def _tile_x(xe):
    """[C, D] fp16 -> [128, DC, C]: [p, dc, t] = xe[t, dc*128+p]."""
    Cc = xe.shape[0]
    return np.ascontiguousarray(xe.T.reshape(DC, 128, Cc).transpose(1, 0, 2))


def _tile_wgwu(w):
    """[H, D] -> [128, HT, DC, 128] fp16: [p,ht,dc,m] = w[ht*128+m, dc*128+p]."""
    return np.ascontiguousarray(
        w.reshape(HT, 128, DC, 128).transpose(3, 0, 2, 1).astype(np.float16))


def _tile_wd(w):
    """[D, H] -> [128, DC, HT, 128] fp16: [p,dt,hc,m] = w[dt*128+m, hc*128+p]."""
    return np.ascontiguousarray(
        w.reshape(DC, 128, HT, 128).transpose(3, 0, 2, 1).astype(np.float16))


def _r2(n):
    return max(8, -(-n // 2) * 2)


def kernel(x, Wr, Wg, Wu, Wd):
    B, S, _ = x.shape
    N = B * S
    xt = np.ascontiguousarray(np.asarray(x, dtype=np.float32).reshape(N, D))
    Wr = np.asarray(Wr, dtype=np.float32)

    logits64, idx, w = _route(xt, Wr)

    # dispatch lists per expert (np.where on [N, K] is token-ordered)
    rows_n, rows_k, counts = [], [], []
    for e in range(E):
        rn, rk = np.nonzero(idx == e)
        rows_n.append(rn)
        rows_k.append(rk)
        counts.append(len(rn))
    counts = np.asarray(counts)

    # pair largest with smallest expert; each pair spans two cores
    order = np.argsort(-counts, kind="stable")
    pairs = [(int(order[i]), int(order[E - 1 - i])) for i in range(E // 2)]
    cA = _r2(max(-(-counts[eb] // 2) for eb, _ in pairs))
    cB = _r2(max(-(-counts[es] // 2) for _, es in pairs))

    nc = _get_kernel(cA, cB)

    xt16 = xt.astype(np.float16)
    wts = {e: (_tile_wgwu(np.asarray(Wg[e])), _tile_wgwu(np.asarray(Wu[e])),
               _tile_wd(np.asarray(Wd[e]))) for e in range(E)}

    # slot assignment: pair p -> cores 2p, 2p+1; core k gets half k of each
    # expert's token list (slot A = big expert, slot B = small expert)
    slot_rows = []  # per core: (rowsA, rowsB) token/slot index arrays
    in_maps = []
    for p, (eb, es) in enumerate(pairs):
        hb = (counts[eb] + 1) // 2
        hs = (counts[es] + 1) // 2
        for k in range(2):
            ra = slice(0, hb) if k == 0 else slice(hb, counts[eb])
            rb = slice(0, hs) if k == 0 else slice(hs, counts[es])
            rnA, rkA = rows_n[eb][ra], rows_k[eb][ra]
            rnB, rkB = rows_n[es][rb], rows_k[es][rb]
            xe = np.zeros((cA + cB, D), dtype=np.float16)
            xe[:len(rnA)] = xt16[rnA]
            xe[cA:cA + len(rnB)] = xt16[rnB]
            gA, uA, dA = wts[eb]
            gB, uB, dB = wts[es]
            in_maps.append({"xT": _tile_x(xe),
                            "wgA": gA, "wuA": uA, "wdA": dA,
                            "wgB": gB, "wuB": uB, "wdB": dB})
            slot_rows.append(((rnA, rkA), (rnB, rkB)))

    global LAST_RUN
    LAST_RUN = run_bass_kernel_spmd(nc, in_maps, list(range(NCORES)))
    results = LAST_RUN.results

    out = np.zeros((N, D), dtype=np.float32)
    for c in range(NCORES):
        y_t = results[c]["y"]                       # [128, DC, cA+cB] f32
        y_tok = y_t.transpose(2, 1, 0).reshape(cA + cB, D)
        for (rn, rk), base in zip(slot_rows[c], (0, cA)):
            if len(rn) == 0:
                continue
            out[rn] += y_tok[base:base + len(rn)] * w[rn, rk][:, None]

    return out.reshape(B, S, D), logits64.astype(np.float32)


# revision 37
# speedup vs baseline: 1.0199x; 1.0057x over previous
"""MoE top-2-of-8 SwiGLU kernel for 8 Trainium2 NeuronCores.

Strategy (expert-parallel with split experts, per sharding hint):
  - Router (tiny: N x E x D matmul) + top-2 dispatch computed on host in
    float64; this IS the sharding step — tokens are gathered per expert id.
  - Each expert's token list is split in half across two cores; each core
    hosts halves of two experts (largest paired with smallest), so the
    per-core capacity is ~max_pair_sum/2 instead of max expert count.
  - Device kernel per core, per expert-slot: h = silu(x@Wg^T) * (x@Wu^T);
    y = h@Wd^T. All matmuls fp16 operands (1 cycle/row on the PE array)
    accumulating in fp32 PSUM. Activations kept transposed
    [feature, token] so the token dim is the moving operand (no 128-row
    quantization of the token count).
  - Host combines: out[n] += w[n,k] * y_row (scatter by the dispatch
    permutation; indices within a slot are unique).

Shapes (hardcoded per problem spec): B=2, S=2048, D=1024, H=4096, E=8, K=2.
"""

import sys

import numpy as np

import concourse.bass as bass
import concourse.tile as tile
from concourse import bacc, mybir
from concourse.bass_utils import run_bass_kernel_spmd


def _ensure_ntff_hook():
    """bass_utils' trace=True path imports antenv.axon_hooks, which some
    agent images lack. Provide the same ctypes shim trn_boot.py would
    install so tracing degrades gracefully instead of crashing."""
    try:
        import antenv.axon_hooks  # noqa: F401
        return
    except ImportError:
        pass
    import contextlib
    import ctypes
    import os
    import types

    so_path = "/opt/axon/libaxon_pjrt.so"
    hook = None
    if os.path.exists(so_path):
        try:
            lib = ctypes.CDLL(so_path)
            if hasattr(lib, "axon_start_nrt_profile"):
                lib.axon_start_nrt_profile.argtypes = [
                    ctypes.POINTER(ctypes.c_int64), ctypes.c_size_t]
                lib.axon_start_nrt_profile.restype = ctypes.c_int64
                lib.axon_stop_nrt_profile.argtypes = [ctypes.c_char_p]
                lib.axon_stop_nrt_profile.restype = ctypes.c_int64

                @contextlib.contextmanager
                def _hook(output_dir, device_ids):
                    import jax
                    jax.devices()
                    if device_ids:
                        ids = (ctypes.c_int64 * len(device_ids))(*device_ids)
                        rc = lib.axon_start_nrt_profile(ids, len(device_ids))
                    else:
                        rc = lib.axon_start_nrt_profile(None, 0)
                    if rc != 0:
                        raise RuntimeError(f"axon_start_nrt_profile rc={rc}")
                    try:
                        yield
                    finally:
                        lib.axon_stop_nrt_profile(str(output_dir).encode())

                hook = _hook
        except OSError:
            hook = None
    mod = types.ModuleType("antenv.axon_hooks")
    mod.get_axon_ntff_profile_hook = lambda: hook
    mod.set_axon_ntff_profile_hook = lambda h: None
    sys.modules["antenv.axon_hooks"] = mod


_ensure_ntff_hook()

TOPK = 2
D = 1024
H = 4096
E = 8
NCORES = 8
DC = D // 128   # 8 contraction chunks of D
HT = H // 128   # 32 tiles of H

_compiled_cache = {}
LAST_RUN = None  # BassKernelResults of the most recent SPMD launch


def _chunk_sizes(C, max_chunk=512):
    """Split C into chunks <= max_chunk (PSUM bank = 512 fp32), multiples
    of 2, near-even."""
    assert C % 2 == 0
    n = -(-C // max_chunk)
    base = -(-C // (2 * n)) * 2
    sizes = []
    left = C
    for _ in range(n):
        s = min(base, left)
        sizes.append(s)
        left -= s
    assert left == 0
    return [s for s in sizes if s]


def _build_kernel(cA, cB, silu_mode="silu"):
    """One SPMD program: two expert slots per core (capacities cA, cB).
    Inputs: xT [128, DC, cA+cB] fp16 (slot A tokens then slot B tokens),
    per-slot pre-tiled weights. Output yT [128, DC, cA+cB] fp32.

    silu_mode="sigmoid_mul" avoids the Silu LUT (not implemented in
    CoreSim) by computing sigmoid on ACT and an extra multiply on DVE.
    """
    C = cA + cB
    parts = [(0, cA, "A"), (cA, cB, "B")]
    f16 = mybir.dt.float16
    f32 = mybir.dt.float32

    nc = bacc.Bacc("TRN2", target_bir_lowering=False, debug=False,
                   num_devices=NCORES)

    xT_d = nc.dram_tensor("xT", [128, DC, C], f16, kind="ExternalInput")
    wdecl = {}
    for _, _, s in parts:
        wdecl["wg" + s] = nc.dram_tensor(
            "wg" + s, [128, HT, DC, 128], f16, kind="ExternalInput")
        wdecl["wu" + s] = nc.dram_tensor(
            "wu" + s, [128, HT, DC, 128], f16, kind="ExternalInput")
        wdecl["wd" + s] = nc.dram_tensor(
            "wd" + s, [128, DC, HT, 128], f16, kind="ExternalInput")
    y_d = nc.dram_tensor("y", [128, DC, C], f32, kind="ExternalOutput")

    with tile.TileContext(nc) as tc:
        with (
            tc.tile_pool(name="xp", bufs=1) as xp,
            tc.tile_pool(name="hp", bufs=1) as hp,
            tc.tile_pool(name="wgp", bufs=3) as wgp,
            tc.tile_pool(name="wup", bufs=3) as wup,
            tc.tile_pool(name="tmp", bufs=3) as tmpp,
            tc.tile_pool(name="outp", bufs=4) as outp,
            tc.tile_pool(name="warm", bufs=1) as warmp,
            tc.tile_pool(name="pa", bufs=2, space="PSUM") as pap,
            tc.tile_pool(name="pb", bufs=2, space="PSUM") as pbp,
            tc.tile_pool(name="pc", bufs=2, space="PSUM") as pcp,
            tc.tile_pool(name="pwarm", bufs=1, space="PSUM") as pwp,
        ):
            # PE warm-up: dummy matmuls on a zeroed scratch tile bridge the
            # initial input-DMA window (~16us, bound by total queued DMA
            # bytes) so the HAM clock-gate opens (1.2->2.4 GHz) before the
            # first real matmul and does not re-cool.
            wsrc = warmp.tile([128, 512], f16)
            nc.gpsimd.memset(wsrc[:], 0)
            pw = pwp.tile([128, 512], f32)
            for _ in range(16):
                nc.tensor.matmul(pw[:], wsrc[:, :128], wsrc[:], start=True,
                                 stop=True)

            # Startup-critical DMAs first: slot-A first weight tiles, then
            # slot-A x split per-dc across sync+gpsimd (SWDGE trigger
            # ~0.6us each). Slot-B x is deferred into the phase-1A loop —
            # it is needed only ~150us in and would otherwise dilute the
            # startup DMA batch (concurrent DMAs complete together).
            wgA0 = wgp.tile([128, DC, 128], f16, tag="w")
            nc.sync.dma_start(wgA0[:], wdecl["wgA"][:, 0])
            wuA0 = wup.tile([128, DC, 128], f16)
            nc.sync.dma_start(wuA0[:], wdecl["wuA"][:, 0])
            xT = xp.tile([128, DC, C], f16)
            h2 = DC // 2
            for dc in range(DC):
                eng = nc.sync if dc % 2 == 0 else nc.gpsimd
                eng.dma_start(xT[:, dc, 0:cA], xT_d[:, dc, 0:cA])
            hT = hp.tile([128, HT, C], f16)

            # Phase 1 per slot: hT[:, ht, slot] = silu(x@Wg^T) * (x@Wu^T).
            # wg and wd share one pool tag: the wd (and slot-B) prefetch
            # DMAs wait on late slot releases instead of stealing HBM
            # bandwidth from the startup-critical loads.
            for base, cap, s in parts:
                wg_d, wu_d = wdecl["wg" + s], wdecl["wu" + s]
                chunks = _chunk_sizes(cap)
                for ht in range(HT):
                    if s == "A" and ht == 8:
                        # deferred slot-B x load, clear of the startup batch
                        nc.sync.dma_start(xT[:, 0:h2, cA:C],
                                          xT_d[:, 0:h2, cA:C])
                        nc.gpsimd.dma_start(xT[:, h2:DC, cA:C],
                                            xT_d[:, h2:DC, cA:C])
                    if s == "A" and ht == 0:
                        wg, wu = wgA0, wuA0
                    else:
                        wg = wgp.tile([128, DC, 128], f16, tag="w")
                        nc.sync.dma_start(wg[:], wg_d[:, ht])
                        wu = wup.tile([128, DC, 128], f16)
                        nc.sync.dma_start(wu[:], wu_d[:, ht])
                    sl = tmpp.tile([128, cap], f32, tag="sl")
                    t0 = base
                    for tn in chunks:
                        pA = pap.tile([128, tn], f32, tag="pA")
                        for dc in range(DC):
                            nc.tensor.matmul(pA[:, 0:tn], wg[:, dc],
                                             xT[:, dc, t0:t0 + tn],
                                             start=(dc == 0),
                                             stop=(dc == DC - 1))
                        pB = pbp.tile([128, tn], f32, tag="pB")
                        for dc in range(DC):
                            nc.tensor.matmul(pB[:, 0:tn], wu[:, dc],
                                             xT[:, dc, t0:t0 + tn],
                                             start=(dc == 0),
                                             stop=(dc == DC - 1))
                        slc = sl[:, t0 - base:t0 - base + tn]
                        if silu_mode == "silu":
                            nc.scalar.activation(
                                slc, pA[:, 0:tn],
                                mybir.ActivationFunctionType.Silu)
                        else:
                            nc.scalar.activation(
                                slc, pA[:, 0:tn],
                                mybir.ActivationFunctionType.Sigmoid)
                            nc.vector.tensor_mul(slc, slc, pA[:, 0:tn])
                        nc.vector.tensor_mul(hT[:, ht, t0:t0 + tn], slc,
                                             pB[:, 0:tn])
                        t0 += tn

            # Phase 2 per slot: y[:, dt, slot] = h @ Wd^T. Slot B first, and
            # slot A's chunking is skewed small-last, so the kernel-tail
            # serial chain (last matmul -> copy -> out-DMA -> drain) runs on
            # the smallest possible chunk.
            for pi, (base, cap, s) in enumerate(parts[::-1]):
                wd_d = wdecl["wd" + s]
                chunks = _chunk_sizes(cap)
                if pi == len(parts) - 1 and len(chunks) > 1:
                    small = -(-min(152, chunks[-1]) // 2) * 2
                    chunks = _chunk_sizes(cap - small) + [small]
                for dt in range(DC):
                    wd = wgp.tile([128, HT, 128], f16, tag="w")
                    nc.sync.dma_start(wd[:], wd_d[:, dt])
                    ot = outp.tile([128, cap], f32, tag="ot")
                    t0 = base
                    for tn in chunks:
                        pC = pcp.tile([128, tn], f32, tag="pC")
                        for hc in range(HT):
                            nc.tensor.matmul(pC[:, 0:tn], wd[:, hc],
                                             hT[:, hc, t0:t0 + tn],
                                             start=(hc == 0),
                                             stop=(hc == HT - 1))
                        oc = ot[:, t0 - base:t0 - base + tn]
                        nc.vector.tensor_copy(oc, pC[:, 0:tn])
                        nc.sync.dma_start(y_d[:, dt, t0:t0 + tn], oc)
                        t0 += tn

    nc.compile()
    return nc


def _get_kernel(cA, cB):
    key = (cA, cB)
    if key not in _compiled_cache:
        _compiled_cache[key] = _build_kernel(cA, cB)
    return _compiled_cache[key]


def _route(xt, Wr):
    """Host router in float64: logits, top-2 (desc, ties by index like
    jax.lax.top_k), renormalized weights."""
    logits64 = xt.astype(np.float64) @ Wr.T.astype(np.float64)
    m = logits64.max(axis=-1, keepdims=True)
    p = np.exp(logits64 - m)
    p /= p.sum(axis=-1, keepdims=True)
    idx = np.argsort(-p, axis=-1, kind="stable")[:, :TOPK]
    w = np.take_along_axis(p, idx, axis=-1)
    w /= w.sum(axis=-1, keepdims=True)
    return logits64, idx, w.astype(np.float32)


def _tile_x(xe):
    """[C, D] fp16 -> [128, DC, C]: [p, dc, t] = xe[t, dc*128+p]."""
    Cc = xe.shape[0]
    return np.ascontiguousarray(xe.T.reshape(DC, 128, Cc).transpose(1, 0, 2))


def _tile_wgwu(w):
    """[H, D] -> [128, HT, DC, 128] fp16: [p,ht,dc,m] = w[ht*128+m, dc*128+p]."""
    return np.ascontiguousarray(
        w.reshape(HT, 128, DC, 128).transpose(3, 0, 2, 1).astype(np.float16))


def _tile_wd(w):
    """[D, H] -> [128, DC, HT, 128] fp16: [p,dt,hc,m] = w[dt*128+m, hc*128+p]."""
    return np.ascontiguousarray(
        w.reshape(DC, 128, HT, 128).transpose(3, 0, 2, 1).astype(np.float16))


def _r2(n):
    return max(8, -(-n // 2) * 2)


def kernel(x, Wr, Wg, Wu, Wd):
    B, S, _ = x.shape
    N = B * S
    xt = np.ascontiguousarray(np.asarray(x, dtype=np.float32).reshape(N, D))
    Wr = np.asarray(Wr, dtype=np.float32)

    logits64, idx, w = _route(xt, Wr)

    # dispatch lists per expert (np.where on [N, K] is token-ordered)
    rows_n, rows_k, counts = [], [], []
    for e in range(E):
        rn, rk = np.nonzero(idx == e)
        rows_n.append(rn)
        rows_k.append(rk)
        counts.append(len(rn))
    counts = np.asarray(counts)

    # pair largest with smallest expert; each pair spans two cores
    order = np.argsort(-counts, kind="stable")
    pairs = [(int(order[i]), int(order[E - 1 - i])) for i in range(E // 2)]
    cA = _r2(max(-(-counts[eb] // 2) for eb, _ in pairs))
    cB = _r2(max(-(-counts[es] // 2) for _, es in pairs))

    nc = _get_kernel(cA, cB)

    xt16 = xt.astype(np.float16)
    wts = {e: (_tile_wgwu(np.asarray(Wg[e])), _tile_wgwu(np.asarray(Wu[e])),
               _tile_wd(np.asarray(Wd[e]))) for e in range(E)}

    # slot assignment: pair p -> cores 2p, 2p+1; core k gets half k of each
    # expert's token list (slot A = big expert, slot B = small expert)
    slot_rows = []  # per core: (rowsA, rowsB) token/slot index arrays
    in_maps = []
    for p, (eb, es) in enumerate(pairs):
        hb = (counts[eb] + 1) // 2
        hs = (counts[es] + 1) // 2
        for k in range(2):
            ra = slice(0, hb) if k == 0 else slice(hb, counts[eb])
            rb = slice(0, hs) if k == 0 else slice(hs, counts[es])
            rnA, rkA = rows_n[eb][ra], rows_k[eb][ra]
            rnB, rkB = rows_n[es][rb], rows_k[es][rb]
            xe = np.zeros((cA + cB, D), dtype=np.float16)
            xe[:len(rnA)] = xt16[rnA]
            xe[cA:cA + len(rnB)] = xt16[rnB]
            gA, uA, dA = wts[eb]
            gB, uB, dB = wts[es]
            in_maps.append({"xT": _tile_x(xe),
                            "wgA": gA, "wuA": uA, "wdA": dA,
                            "wgB": gB, "wuB": uB, "wdB": dB})
            slot_rows.append(((rnA, rkA), (rnB, rkB)))

    global LAST_RUN
    LAST_RUN = run_bass_kernel_spmd(nc, in_maps, list(range(NCORES)))
    results = LAST_RUN.results

    out = np.zeros((N, D), dtype=np.float32)
    for c in range(NCORES):
        y_t = results[c]["y"]                       # [128, DC, cA+cB] f32
        y_tok = y_t.transpose(2, 1, 0).reshape(cA + cB, D)
        for (rn, rk), base in zip(slot_rows[c], (0, cA)):
            if len(rn) == 0:
                continue
            out[rn] += y_tok[base:base + len(rn)] * w[rn, rk][:, None]

    return out.reshape(B, S, D), logits64.astype(np.float32)
